# revision 81
# baseline (speedup 1.0000x reference)
"""Trainium2 Bass kernel for nn_Memory (topk_masking).

Algorithm (per query row q of N=32768, item count 2048, K=10):
  logits l = q @ mempool.T
  e = exp(l); S = sum(e)
  top-10 threshold t10 via chunked DVE max8 (8 chunks of 256 -> 64
  candidates, then max8 / match_replace / max8 on the candidates)
  u = exp(e/S);  g = (e >= t10) * u;  Z = sum(g)
  out = (g @ mempool) / Z

Precision: mm1 is computed as 2^17*(qh@mh + qh@ml + ql@mh) where qh/mh are
fp16 splits of q/mempool^T:
  - qh@mh as a plain fp16 matmul (exact: operands are fp16 values by
    construction, accumulation is fp32 in PSUM). Power-of-2 pre-scales
    (2^9*qh, 2^8*mh) keep fp16 representability.
  - qh@ml + ql@mh in ONE fp8e4 DoubleRow matmul (0.5 cycles/row): pairs
    (2^4*qh, 2^13*ml) and (2^13*ql, 2^4*mh) both carry product scale 2^17.
  The 2^17 factor is folded out via the exp activation's scale argument.
  Residual logit error sigma ~7e-6 -> ~6 of 32768 rows flip top-10
  selection vs the fp32 reference; output rel err ~7e-3 (fp16 mm2 floor
  ~3e-4).

Transpose of g for mm2 runs on the tensor engine (16 permutation matmuls,
is_transpose=True) + a DVE copy to SBUF. dma_start_transpose is NOT used:
its completion accounting is unreliable in this stack and corrupts
pipeline-warmup tiles.

Sharding: data-parallel over queries. 32 units of [512 dim x 1024 queries]
(16 batches x 2 inputs); each of 8 cores takes 4 units = 32 tiles of 128
queries. mempool (fp16+fp8 copies, ~6MB) replicated per core, streamed in
first-use order so warmup matmuls start after ~0.5MB.

Pipelined emission: iteration t issues front(t) (mm1..g), then mid(t-1)
(PE transpose + copy), then back(t-PIPE) (mm2, scale, store), so the PE
stream is mm1(t), tr(t-1), mm2(t-PIPE) and never waits on the g-chain.
"""
import sys
sys.path.insert(0, '/opt/trn_rl_repo')

import numpy as np
import concourse.bacc as bacc
import concourse.mybir as mybir
import concourse.tile as tile
from concourse.bass_utils import run_bass_kernel_spmd

F32 = mybir.dt.float32
F32R = mybir.dt.float32r
F16 = mybir.dt.float16
F8 = mybir.dt.float8e4

DIM = 512
NITEM = 2048
NCORES = 8
UNITS_PER_CORE = 4
QPU = 1024
TILES = UNITS_PER_CORE * QPU // 128
NEG = -1e30
EXP = mybir.ActivationFunctionType.Exp
SIGMOID = mybir.ActivationFunctionType.Sigmoid
DR = mybir.MatmulPerfMode.DoubleRow
NCHUNK = 8                      # max8 chunks over items
CHUNK = NITEM // NCHUNK         # 256
PIPE = 2                        # software-pipeline depth (front t .. back t-PIPE)
C_SOFT = 4e-6                   # soft-mask transition width (~0.6x logit noise)
LSCALE = 2.0 ** 17              # logit scale carried through mm1

_prog_cache = {}


def declare_io(nc):
    decl = lambda n, s, d: nc.declare_dram_parameter(n, s, d, isOutput=False)
    return {
        "qm": decl("qm", [TILES, 128, DIM], F16),             # 2^9 * qh
        "q8": decl("q8", [TILES, 128, 4 * 2 * 128], F8),      # (2^4 qh, 2^13 ql)
        "m32": decl("m32", [DIM, NITEM], F16),                # 2^8 * mh
        "m8": decl("m8", [DIM, 2, NITEM], F8),                # (2^13 ml, 2^4 mh)
        "mp": decl("mp", [NITEM, DIM], F16),
        "ident": decl("ident", [128, 128], F16),
        "konst": decl("konst", [128, 1], F32),                # 1/LSCALE
        "out": nc.declare_dram_parameter("out", [UNITS_PER_CORE * QPU, DIM],
                                         F32, isOutput=True),
    }


def emit(nc, tc, dram):
    with (
        tc.tile_pool(name="const", bufs=1) as cpool,
        tc.tile_pool(name="qin", bufs=4) as qpool,
        tc.tile_pool(name="work", bufs=2) as wpool,
        tc.tile_pool(name="epool", bufs=3) as epool,
        tc.tile_pool(name="uspool", bufs=2) as uspool,
        tc.tile_pool(name="gp", bufs=3) as gpool,
        tc.tile_pool(name="gtp", bufs=5) as gtpool,
        tc.tile_pool(name="zp", bufs=6) as zpool,
        tc.tile_pool(name="outp", bufs=4) as opool,
        tc.tile_pool(name="ps_l", bufs=1, space="PSUM") as ps_l,
        tc.tile_pool(name="ps_l2", bufs=2, space="PSUM") as ps_l2,
        tc.tile_pool(name="ps_t", bufs=1, space="PSUM") as ps_t,
        tc.tile_pool(name="ps_o", bufs=1, space="PSUM") as ps_o,
    ):
        # constants split per kc-chunk and DMA-ordered by first use: chunk 0,
        # then tile-0's q loads, then the remaining chunks, so the first
        # matmuls start after ~1.5MB instead of stalling ~27us for the full
        # 8MB; mm2-only constants (ident, mp) stream in last
        k_sb = cpool.tile([128, 1], F32)
        nc.sync.dma_start(k_sb[:], dram["konst"][:])
        m_t = [[cpool.tile([128, 512], F16, name=f"m_sb{kc}_{b}") for b in range(4)]
               for kc in range(4)]
        m8_kc = [cpool.tile([128, 2, NITEM], F8, name=f"m8_sb{kc}") for kc in range(4)]
        id_sb = cpool.tile([128, 128], F16)
        mp_sb = cpool.tile([128, 16, DIM], F16)

        def load_q(t):
            q_sb = qpool.tile([128, 4, 128], F16, tag="qx", name="q_sb")
            nc.sync.dma_start(q_sb[:], dram["qm"][t]
                              .rearrange("p (kc f) -> p kc f", kc=4))
            q8_sb = qpool.tile([128, 4, 2, 128], F8, tag="q8", name="q8_sb")
            nc.sync.dma_start(q8_sb[:], dram["q8"][t]
                              .rearrange("p (kc two f) -> p kc two f", kc=4, two=2))
            return q_sb, q8_sb

        preq0 = load_q(0)
        for kc in range(4):
            for b in range(4):
                nc.sync.dma_start(m_t[kc][b][:],
                                  dram["m32"][128 * kc:128 * (kc + 1),
                                              512 * b:512 * (b + 1)])
            nc.sync.dma_start(m8_kc[kc][:], dram["m8"][128 * kc:128 * (kc + 1), :, :])

        state = {}              # per-tile tiles needed by later stages

        def front(t):
            q_sb, q8_sb = preq0 if t == 0 else load_q(t)

            e_sb = epool.tile([128, NITEM], F32, tag="e", name="e_sb")
            S_p = wpool.tile([128, 4], F32, tag="Sp", name="S_p")
            # mm1 in 4 item-blocks of 512, each in its own PSUM bank so each
            # block's exp overlaps the next block's matmuls; block 0 is
            # double-buffered so the next tile's first matmul never waits on
            # this tile's exp of block 0
            lps = [(ps_l2 if blk == 0 else ps_l).tile(
                [128, 512], F32, tag=f"l{blk}", name=f"l_ps{blk}")
                for blk in range(4)]

            def main_mm(kc, blk):
                nc.tensor.matmul(lps[blk][:], q_sb[:, kc, :], m_t[kc][blk][:],
                                 start=(kc == 0), stop=False)

            def dr_mm(kc, blk):
                sl = slice(512 * blk, 512 * (blk + 1))
                nc.tensor.matmul(lps[blk][:], q8_sb[:, kc], m8_kc[kc][:, :, sl],
                                 start=False, stop=(kc == 3), perf_mode=DR)

            def exp_blk(blk):
                nc.scalar.activation(e_sb[:, 512 * blk:512 * (blk + 1)],
                                     lps[blk][:], EXP, scale=k_sb[:],
                                     accum_out=S_p[:, blk:blk + 1])

            if t == 0:
                # warmup: kc-outer so matmuls start as constant chunks land
                for kc in range(4):
                    for blk in range(4):
                        main_mm(kc, blk)
                for kc in range(4):
                    for blk in range(4):
                        dr_mm(kc, blk)
                for blk in range(4):
                    exp_blk(blk)
            else:
                for blk in range(4):
                    for kc in range(4):
                        main_mm(kc, blk)
                    for kc in range(4):
                        dr_mm(kc, blk)
                    exp_blk(blk)

            S01 = wpool.tile([128, 2], F32, tag="S01", name="S01")
            nc.vector.tensor_add(S01[:, 0:1], S_p[:, 0:1], S_p[:, 1:2])
            nc.vector.tensor_add(S01[:, 1:2], S_p[:, 2:3], S_p[:, 3:4])
            Sinv = wpool.tile([128, 1], F32, tag="Sinv", name="Sinv")
            nc.vector.tensor_add(Sinv[:], S01[:, 0:1], S01[:, 1:2])
            nc.vector.reciprocal(Sinv[:], Sinv[:])

            # chunked top-k: top-8 of each 256-chunk -> 64 candidates
            cand = wpool.tile([128, NCHUNK, 8], F32, tag="cand", name="cand")
            for c in range(NCHUNK):
                nc.vector.max(out=cand[:, c, :],
                              in_=e_sb[:, CHUNK * c:CHUNK * (c + 1)])
            cflat = cand[:].rearrange("p c k -> p (c k)")
            top8 = wpool.tile([128, 8], F32, tag="top8", name="top8")
            candm = wpool.tile([128, NCHUNK * 8], F32, tag="candm", name="candm")
            next8 = wpool.tile([128, 8], F32, tag="next8", name="next8")
            nc.vector.max(out=top8[:], in_=cflat)
            nc.vector.match_replace(out=candm[:], in_to_replace=top8[:],
                                    in_values=cflat, imm_value=NEG)
            nc.vector.max(out=next8[:], in_=candm[:])

            u_sb = uspool.tile([128, NITEM], F16, tag="u", name="u_sb")
            nc.scalar.activation(u_sb[:], e_sb[:], EXP, scale=Sinv[:])

            # hard top-10 mask: with logit noise sigma ~7e-6, only ~5 rows
            # in 32768 flip selection vs the fp32 reference (~5e-3 rel err);
            # a sigmoid soft mask would force Exp<->Sigmoid activation-table
            # reloads (~2.6us/tile on ACT) for a ~1e-3 rel improvement
            g_sb = gpool.tile([128, NITEM], F16, tag="g", name="g_sb")
            Z_sb = zpool.tile([128, 1], F32, tag="Z", name="Z_sb")
            nc.vector.scalar_tensor_tensor(
                out=g_sb[:], in0=e_sb[:], scalar=next8[:, 1:2], in1=u_sb[:],
                op0=mybir.AluOpType.is_ge, op1=mybir.AluOpType.mult,
                accum_out=Z_sb[:])
            state[t] = (g_sb, Z_sb)

        def mid(t):
            # transpose g on the tensor engine (16 permutation matmuls into
            # PSUM f16) + DVE copy to SBUF
            g_sb, Z_sb = state.pop(t)
            gt_ps = ps_t.tile([128, 16, 128], F16, tag="gtp", name="gt_ps")
            for ic in range(16):
                nc.tensor.matmul(gt_ps[:, ic, :], g_sb[:, 128 * ic:128 * (ic + 1)],
                                 id_sb[:], is_transpose=True)
            gt_sb = gtpool.tile([128, 16, 128], F16, tag="gt", name="gt_sb")
            nc.vector.tensor_copy(gt_sb[:], gt_ps[:])
            state[t] = (gt_sb, Z_sb)

        def back(t):
            gt_sb, Z_sb = state.pop(t)
            o_ps = ps_o.tile([128, DIM], F32, tag="o", name="o_ps")
            for ic in range(16):
                nc.tensor.matmul(o_ps[:], gt_sb[:, ic, :], mp_sb[:, ic, :],
                                 start=(ic == 0), stop=(ic == 15))
            # Zinv here (not in front): a reciprocal emitted in front would
            # head-of-line block the next tile's DVE work
            Zinv = zpool.tile([128, 1], F32, tag="Zinv", name="Zinv")
            nc.vector.reciprocal(Zinv[:], Z_sb[:])
            o_sb = opool.tile([128, DIM], F32, tag="osb", name="o_sb")
            nc.vector.tensor_scalar_mul(o_sb[:], o_ps[:], Zinv[:])
            nc.sync.dma_start(dram["out"][128 * t:128 * (t + 1), :], o_sb[:])

        for t in range(TILES + PIPE):
            if t < TILES:
                front(t)
            if t == 0:
                nc.sync.dma_start(id_sb[:], dram["ident"][:])
            if t == 1:
                nc.sync.dma_start(mp_sb[:], dram["mp"][:]
                                  .rearrange("(ic p) d -> p ic d", p=128))
            if 1 <= t <= TILES:
                mid(t - 1)
            if t >= PIPE:
                back(t - PIPE)


def build_program():
    if 'nc' in _prog_cache:
        return _prog_cache['nc']
    nc = bacc.Bacc()
    dram = declare_io(nc)
    with tile.TileContext(nc) as tc:
        emit(nc, tc, dram)
    nc.finalize()
    _prog_cache['nc'] = nc
    return nc


def _prep_inputs(input1, input2, mempool):
    from ml_dtypes import float8_e4m3fn as f8

    units = np.concatenate([
        np.asarray(input1, dtype=np.float32).reshape(16, DIM, QPU),
        np.asarray(input2, dtype=np.float32).reshape(16, DIM, QPU),
    ], axis=0)                                     # [32, 512, 1024]
    uh = units.astype(np.float16).astype(np.float32)
    ul = units - uh

    # [unit, kc, p, tt, f] -> [unit, tt, p, (kc f)]: tile row (2KB) contiguous
    qm4 = (uh * 2.0 ** 9).astype(np.float16).reshape(32, 4, 128, 8, 128)
    qm_all = np.ascontiguousarray(
        qm4.transpose(0, 3, 2, 1, 4).reshape(32, 8, 128, 512))
    # fp8 pairs: [unit, kc, two, p, tt, f] -> [unit, tt, p, (kc two f)]
    q8_pair = np.stack([(uh * 2.0 ** 4).reshape(32, 4, 128, QPU),
                        (ul * 2.0 ** 13).reshape(32, 4, 128, QPU)], axis=2)
    q8_l = q8_pair.reshape(32, 4, 2, 128, 8, 128).transpose(0, 4, 3, 1, 2, 5)
    q8_all = np.ascontiguousarray(q8_l.reshape(32, 8, 128, 4 * 2 * 128)).astype(f8)

    mpT = np.ascontiguousarray(np.asarray(mempool, dtype=np.float32).T)  # [512, 2048]
    mh = mpT.astype(np.float16).astype(np.float32)
    ml = mpT - mh
    m32 = (mh * 2.0 ** 8).astype(np.float16)
    m8 = np.ascontiguousarray(
        np.stack([ml * 2.0 ** 13, mh * 2.0 ** 4], axis=1)).astype(f8)  # [512, 2, 2048]

    mp16 = np.asarray(mempool, dtype=np.float32).astype(np.float16)
    ident = np.eye(128, dtype=np.float16)
    konst = np.full((128, 1), 1.0 / LSCALE, dtype=np.float32)
    return [{
        "qm": qm_all[4 * k:4 * (k + 1)].reshape(TILES, 128, DIM),
        "q8": q8_all[4 * k:4 * (k + 1)].reshape(TILES, 128, 4 * 2 * 128),
        "m32": m32, "m8": m8, "mp": mp16, "ident": ident, "konst": konst,
    } for k in range(NCORES)]


def _assemble(results):
    outs = np.empty((32, DIM, QPU), dtype=np.float32)
    for k in range(NCORES):
        o = results[k]["out"]
        for j in range(UNITS_PER_CORE):
            outs[4 * k + j] = o[QPU * j:QPU * (j + 1), :].T
    return outs[:16].reshape(16, DIM, 32, 32), outs[16:].reshape(16, DIM, 32, 32)


def kernel(input1, input2, mempool):
    nc = build_program()
    in_maps = _prep_inputs(input1, input2, mempool)
    res = run_bass_kernel_spmd(nc, in_maps, core_ids=list(range(NCORES)))
    return _assemble(res.results)


if __name__ == "__main__":
    rng = np.random.default_rng(0)
    i1 = rng.standard_normal((16, DIM, 32, 32)).astype(np.float32)
    i2 = rng.standard_normal((16, DIM, 32, 32)).astype(np.float32)
    mp = rng.uniform(-1 / np.sqrt(DIM), 1 / np.sqrt(DIM), (NITEM, DIM)).astype(np.float32)
    o1, o2 = kernel(i1, i2, mp)
    print("ok", o1.shape, o2.shape, o1.dtype)
